# revision 58
# baseline (speedup 1.0000x reference)
"""Trainium2 Bass kernel for an 8-layer dense transformer (CloudTransformerMM).

Strategy: data-parallel over tokens (zigzag chunk pairing: core c owns chunks
{c, 15-c} of each batch) across 8 cores, per-layer K/V AllGather.
Feature-major residual stream [D, tokens] per core so projections need no
activation transposes.

v2: bf16 pre-transposed weights, k-major attention (K^T.Q scores, ones-column
softmax denominators, post-PV broadcast normalize), bf16 gathers.

v3+ changes vs v2:
- K+V packed into one AllGather per 128-token block (4/layer). Token order is
  [b0 low-chunk, b1 low, b0 high, b1 high], so each gather lands just before
  its consumers: qi=0 windows need only the low-chunk gathers, and qi=1's
  high-chunk slots run last (v2 stalled ~107us/layer on gather latency).
- Gather payloads laid out so Kg/Vg SBUF loads are large DMAs with >=520B
  contiguous runs (v2: 32 small strided DMAs per layer).
- Attention exp in slot pairs ([128,1024] per ACT op) to amortize the ~300ns
  fixed ACT cost; own-chunk slot unpaired so it needs no gather. Sibling
  (g even, g odd) head-groups are interleaved pair-step by pair-step so PE
  and ACT always hold independent work while MM->exp->PV dependencies resolve.
- rmsnorm: mean+eps folded into ACT sqrt's affine pre-scale, approximate DVE
  reciprocal (~51 ULP), squares alternated ACT/DVE.
- Coalesced weight streams (wq/wo 1x2MB per layer, w1+w3 2MB pairs, w2 512KB
  pairs with 16-deep accumulation chains, lm-head embeddings 2MB pairs,
  single merged logits store per vocab chunk) - per-DMA fixed cost through
  the tunnel-measured ~1-2us made 1029 DMAs/run a real tax (now ~505).
- bf16 logits output (halves the store; rel-err impact ~0.2% in quadrature).
Precision: bf16 weights x f32r activations on PE, bf16 score path, fp32
PSUM + residual.

Measured (8-core TRN2, steady-state pipelined-dispatch slope): 4.74 ms at
round 2; TimelineSim cost model 3.85 ms for the current build.
"""
import math
import sys

sys.path.insert(0, '/opt/trn_rl_repo')

import numpy as np
import ml_dtypes

B, S, D = 2, 2048, 1024
NH, KVH, HD = 16, 4, 64
L, DFF, V = 8, 4096, 32000
THETA, YSCALE, YALPHA, YBETA = 10000.0, 40.0, 1.0, 32.0
ROPE_MAX = 2048 * 40
EPS = 1e-6

NC = 8
NCH = 16
CH = S // NCH          # 128
TPC = 2 * 2 * CH       # 512
KS = D // 128          # 8
NEG = -1.0e30
NVCH = (V + 511) // 512  # 63 (last chunk zero-padded host-side)
KPAY = 64 * 4 * 128      # K payload elems per (b, half) in the kv gather
VPAY = 128 * 4 * 65      # V payload elems (includes the ones column)
PAY = KPAY + VPAY

bf16 = ml_dtypes.bfloat16


def rope_tables():
    inv_freq = 1.0 / THETA ** (np.arange(0, HD, 2, dtype=np.float32) / HD)
    wavelengths = 2.0 * math.pi / inv_freq
    r = ROPE_MAX / wavelengths
    gamma = np.clip((r - YALPHA) / (YBETA - YALPHA), 0.0, 1.0)
    inv_freq = inv_freq * ((1.0 - gamma) / YSCALE + gamma)
    t = np.arange(S, dtype=np.float32)
    freqs = np.outer(t, inv_freq)
    emb = np.concatenate([freqs, freqs], axis=-1)
    emb = emb / math.sqrt(0.1 * math.log(YSCALE) + 1.0)
    return np.cos(emb).astype(np.float32), np.sin(emb).astype(np.float32)


def core_chunks(c):
    return [c, NCH - 1 - c]


def chunk_owner(lk):
    oc = min(lk, NCH - 1 - lk)
    return oc, (0 if lk == oc else 1)


_NC_CACHE = None


def build_nc(no_coll=False, skip_layers=False, skip_lm=False):
    """no_coll / skip_layers / skip_lm build timing-shape variants for
    performance attribution experiments only, never for real output."""
    global _NC_CACHE
    variant = no_coll or skip_layers or skip_lm
    if _NC_CACHE is not None and not variant:
        return _NC_CACHE
    import concourse.mybir as mybir
    import concourse.tile as tile
    from concourse import bacc

    f32 = mybir.dt.float32
    f32r = mybir.dt.float32r
    bfl = mybir.dt.bfloat16
    AF = mybir.ActivationFunctionType
    ALU = mybir.AluOpType

    nc = bacc.Bacc("TRN2", target_bir_lowering=False, debug=False,
                   enable_asserts=True, num_devices=NC)

    def din(name, shape, dt):
        return nc.dram_tensor(name, shape, dt, kind="ExternalInput").ap()

    x0T_d = din("x0T", [128, KS, TPC], f32)
    wq_d = din("wqT", [L * KS, 128, KS, 128], bfl)     # [l*8+mb]
    wk_d = din("wkT", [L, 128, KS, KVH * HD], bfl)
    wv_d = din("wvT", [L, 128, KS, KVH * HD], bfl)
    wo_d = din("woT", [L * KS, 128, KS, 128], bfl)     # [l*8+mb]
    w13_d = din("w13T", [L * 8, 2, 128, KS, 512], bfl)  # [l*8+q8][w1|w3]
    w2_d = din("w2T", [L * 2 * KS, 128, 2, 8, 128], bfl)  # [(l*2+qp)*8+mb]
    n1_d = din("n1", [L, 128, KS], f32)
    n2_d = din("n2", [L, 128, KS], f32)
    fnw_d = din("fnw", [128, KS], f32)
    emb_d = din("embT", [NVCH, 128, KS, 512], bfl)
    cosq_d = din("cosq", [HD, TPC], f32)
    sinq_d = din("sinq", [HD, TPC], f32)
    cosk_d = din("cosk", [HD, TPC], f32)
    sink_d = din("sink", [HD, TPC], f32)
    p64_d = din("p64", [HD, HD], f32r)
    trilT_d = din("trilT", [128, 128], f32)
    ones_d = din("ones128", [128, 128], f32r)
    kbias_d = din("kbias", [2, 2, 8, KVH, 128], bfl)  # [half, qi, slot, g, m]
    qflag_d = din("qflag", [2, NH, TPC], bfl)
    out_d = nc.dram_tensor("out", [TPC, V], bfl, kind="ExternalOutput").ap()

    with tile.TileContext(nc) as tc, \
         tc.tile_pool(name="pers", bufs=1) as pers:
        hT = pers.tile([128, KS, TPC], f32, tag="hT", name="hT")
        qrot = pers.tile([128, NH, TPC], bfl, tag="qrot", name="qrot")
        cosq = pers.tile([HD, TPC], f32, tag="cosq", name="cosq")
        sinq = pers.tile([HD, TPC], f32, tag="sinq", name="sinq")
        cosk = pers.tile([HD, TPC], f32, tag="cosk", name="cosk")
        sink = pers.tile([HD, TPC], f32, tag="sink", name="sink")
        p64 = pers.tile([HD, HD], f32r, tag="p64", name="p64")
        trilT = pers.tile([128, 128], f32, tag="trilT", name="trilT")
        ones128 = pers.tile([128, 128], f32r, tag="ones128", name="ones128")
        eps128 = pers.tile([128, 1], f32, tag="eps128", name="eps128")
        nc.vector.memset(eps128[:], float(EPS))
        # persistent gathered-K/V tiles: kbias rows (64:66) are layer-
        # invariant, loaded once; only rows 0:64 / V data stream per layer
        Kg = [pers.tile([66, 8, KVH, 128], bfl, tag=f"Kg{q}", name=f"Kg{q}")
              for q in range(4)]
        Vg = [pers.tile([128, 8, KVH, HD + 1], bfl, tag=f"Vg{q}",
                        name=f"Vg{q}") for q in range(4)]
        for q in range(4):
            nc.sync.dma_start(Kg[q][64:66], kbias_d[q // 2])

        nc.sync.dma_start(hT[:], x0T_d[:])
        nc.sync.dma_start(qrot[64:66, :, :], qflag_d[:])
        for t_, d_ in ((cosq, cosq_d), (sinq, sinq_d), (cosk, cosk_d),
                       (sink, sink_d), (p64, p64_d), (trilT, trilT_d),
                       (ones128, ones_d)):
            nc.sync.dma_start(t_[:], d_[:])

        def rmsnorm(P, smp, src, w_sb, dst):
            # inv = rsqrt(mean(x^2)+eps): ACT sqrt (with mean+eps folded into
            # its affine pre-scale) + fast approximate DVE reciprocal
            # (~51 ULP, ~5x faster than the bit-exact iterative divide).
            ssp = P.tile([128, TPC], f32, tag="mm", name="ssp")
            for sub in range(KS):
                sq = smp.tile([128, TPC], f32r, tag="sq", name="sq")
                # alternate ACT/DVE so the square stream isn't single-engine
                if sub % 2 == 0:
                    nc.scalar.activation(sq[:], src[:, sub, :], AF.Square)
                else:
                    nc.vector.tensor_mul(sq[:], src[:, sub, :], src[:, sub, :])
                nc.tensor.matmul(ssp[:], ones128[:], sq[:],
                                 start=(sub == 0), stop=(sub == KS - 1))
            sd = smp.tile([128, TPC], f32, tag="sd", name="sd")
            nc.scalar.activation(sd[:], ssp[:], AF.Sqrt, eps128[:], 1.0 / D)
            inv = smp.tile([128, TPC], f32, tag="inv", name="inv")
            nc.vector.reciprocal_approx_fast(inv[:], sd[:])
            for sub in range(KS):
                nc.vector.scalar_tensor_tensor(
                    dst[:, sub, :], src[:, sub, :], w_sb[:, sub:sub + 1],
                    inv[:], ALU.mult, ALU.mult)

        with tc.tile_pool(name="dram", bufs=2, space="DRAM") as dram:

            for l in range([0, L][not skip_layers]):
                with tc.tile_pool(name="layerp", bufs=1) as lp:
                    xn = lp.tile([128, KS, TPC], bfl, tag="xn", name="xn")
                    # block-major: [p, block, g, m] so gather packs read 1KB runs
                    kr = lp.tile([64, 4, KVH, 128], bfl, tag="kr", name="kr")
                    v_s = lp.tile([128, 4, KVH, HD + 1], bfl, tag="v_s",
                                  name="v_s")
                    o_sb = lp.tile([128, KS, TPC], bfl, tag="o", name="o_sb")

                    # ======== phase A: norm1, k/v/q proj + rope + gathers ====
                    kv_out = []
                    with tc.tile_pool(name="phA", bufs=2) as pa, \
                         tc.tile_pool(name="PpsA", bufs=4,
                                      space="PSUM") as ppa:
                        n1sb = pa.tile([128, KS], f32, tag="nw", name="n1sb")
                        nc.sync.dma_start(n1sb[:], n1_d[l])
                        rmsnorm(ppa, pa, hT, n1sb, xn)

                        # k projection + rope (first, to hide gather latency)
                        wk_t = pa.tile([128, KS, KVH * HD], bfl, tag="wkv",
                                       name="wk_t")
                        nc.sync.dma_start(wk_t[:], wk_d[l])
                        k_s = pa.tile([64, KVH, TPC], f32r, tag="k_s", bufs=1,
                                      name="k_s")
                        for mb in range(2):
                            pk = ppa.tile([128, TPC], f32, tag="mm", name="pk")
                            for k in range(KS):
                                nc.tensor.matmul(
                                    pk[:], wk_t[:, k, mb * 128:(mb + 1) * 128],
                                    xn[:, k, :], start=(k == 0),
                                    stop=(k == KS - 1))
                            nc.scalar.copy(k_s[:, 2 * mb, :], pk[0:64, :])
                            nc.scalar.copy(k_s[:, 2 * mb + 1, :], pk[64:128, :])
                        for g in range(KVH):
                            psh = ppa.tile([64, TPC], f32, tag="mm", name="psh")
                            nc.tensor.matmul(psh[:], p64[:], k_s[:, g, :],
                                             start=True, stop=True)
                            tA = pa.tile([64, TPC], f32, tag="tA", name="tA")
                            nc.vector.tensor_mul(tA[:], psh[:], sink[:])
                            tB = pa.tile([64, TPC], f32, tag="tB", name="tB")
                            nc.vector.tensor_mul(tB[:], k_s[:, g, :], cosk[:])
                            nc.vector.tensor_add(
                                kr[:, :, g, :],
                                tA[:].rearrange("p (q m) -> p q m", q=4),
                                tB[:].rearrange("p (q m) -> p q m", q=4))

                        # v projection (token-major, bf16, +ones col)
                        wv_t = pa.tile([128, KS, KVH * HD], bfl, tag="wkv",
                                       name="wv_t")
                        nc.sync.dma_start(wv_t[:], wv_d[l])
                        nc.vector.memset(v_s[:, :, :, HD:HD + 1], 1.0)
                        # combined K+V gathers, one per 128-token block,
                        # each issued as soon as its block's V is ready.
                        # Token order is [b0 low, b1 low, b0 high, b1 high],
                        # so quarter q carries (half X=q//2, batch b=q%2) and
                        # lands just before its consumers need it.
                        for tb in range(4):
                            pv_ = ppa.tile([128, KVH * HD], f32, tag="mm",
                                         name="pv_")
                            for k in range(KS):
                                nc.tensor.matmul(
                                    pv_[:], xn[:, k, tb * 128:(tb + 1) * 128],
                                    wv_t[:, k, :], start=(k == 0),
                                    stop=(k == KS - 1))
                            nc.scalar.copy(
                                v_s[:, tb, :, 0:HD],
                                pv_[:].rearrange("p (g h) -> p g h", g=KVH))
                            kv_in = dram.tile([PAY], bfl, tag="kv_i", bufs=4,
                                              name=f"kv_in{tb}")
                            nc.sync.dma_start(
                                kv_in[0:KPAY].rearrange(
                                    "(p g m) -> p g m", p=64, g=KVH),
                                kr[:, tb])
                            nc.sync.dma_start(
                                kv_in[KPAY:PAY].rearrange(
                                    "(p g h) -> p g h", p=128, g=KVH),
                                v_s[:, tb, :, :])
                            ko = dram.tile([NC, PAY], bfl, tag="kv_o", bufs=4,
                                           addr_space="Shared",
                                           name=f"kv_out{tb}")
                            if not no_coll:
                                nc.gpsimd.collective_compute(
                                    "AllGather", ALU.bypass,
                                    replica_groups=[list(range(NC))],
                                    ins=[kv_in.opt()], outs=[ko.opt()])
                            kv_out.append(ko)

                        # q projection + rope (one 2MB weight load)
                        wq_t = pa.tile([128, KS, KS, 128], bfl, tag="wqo",
                                       name="wq_t")
                        nc.sync.dma_start(
                            wq_t[:],
                            wq_d[l * KS:(l + 1) * KS].rearrange(
                                "b p k m -> p b k m"))
                        for mb in range(KS):
                            pq = ppa.tile([128, TPC], f32, tag="mm", name="pq")
                            for k in range(KS):
                                nc.tensor.matmul(pq[:], wq_t[:, mb, k, :],
                                                 xn[:, k, :], start=(k == 0),
                                                 stop=(k == KS - 1))
                            q_s = pa.tile([64, 2, TPC], f32r, tag="q_s",
                                          name="q_s")
                            nc.scalar.copy(q_s[:, 0, :], pq[0:64, :])
                            nc.scalar.copy(q_s[:, 1, :], pq[64:128, :])
                            for hh in range(2):
                                h_ = 2 * mb + hh
                                psh = ppa.tile([64, TPC], f32, tag="mm",
                                             name="pshq")
                                nc.tensor.matmul(psh[:], p64[:], q_s[:, hh, :],
                                                 start=True, stop=True)
                                tA = pa.tile([64, TPC], f32, tag="tA",
                                             name="tAq")
                                nc.vector.tensor_mul(tA[:], psh[:], sinq[:])
                                tB = pa.tile([64, TPC], f32, tag="tB",
                                             name="tBq")
                                nc.vector.tensor_mul(tB[:], q_s[:, hh, :],
                                                     cosq[:])
                                nc.vector.tensor_add(qrot[0:64, h_, :],
                                                     tA[:], tB[:])

                    # ======== phase B: attention (k-major, A/B halves) =======
                    # Gather half 0 delivers chunks 0..7 (slot s = chunk s),
                    # half 1 delivers chunks 15..8 (slot s = chunk 15-s).
                    # qi=0 windows need only half 0; qi=1 needs both, and its
                    # half-1 slots run last so gather 1 latency hides behind
                    # the whole qi=0 pass.
                    with tc.tile_pool(name="phB", bufs=2) as pb, \
                         tc.tile_pool(name="Psc", bufs=2, space="PSUM") as Psc, \
                         tc.tile_pool(name="Ppv", bufs=2, space="PSUM") as Ppv, \
                         tc.tile_pool(name="Pbc", bufs=2, space="PSUM") as Pbc:
                        for q in range(4):
                            nc.sync.dma_start(
                                Kg[q][0:64],
                                kv_out[q][:, 0:KPAY].rearrange(
                                    "c (p g m) -> p c g m", p=64, g=KVH))
                            nc.sync.dma_start(
                                Vg[q][:],
                                kv_out[q][:, KPAY:PAY].rearrange(
                                    "c (p g h) -> p c g h", p=128, g=KVH))

                        # slot sequences: None = own chunk (local kr/v_s);
                        # (X, s) = gathered half X slot s. Window slots are
                        # exp'd in pairs to amortize ACT fixed cost.
                        SEQ0 = [[None], [(0, 0), (0, 1)], [(0, 2), (0, 3)],
                                [(0, 4), (0, 5)], [(0, 6)]]
                        SEQ1 = [[None], [(0, 0), (0, 1)], [(0, 2), (0, 3)],
                                [(0, 4), (0, 5)], [(0, 6), (0, 7)],
                                [(1, 1), (1, 2)], [(1, 3), (1, 4)],
                                [(1, 5), (1, 6)], [(1, 7)]]
                        # groups (qi, b, g) are processed in sibling pairs
                        # (g even, g odd) with their score/exp/PV pair-steps
                        # interleaved, so PE and ACT always have the other
                        # group's independent work while a cross-engine
                        # dependency (MM -> exp -> PV) resolves.
                        def emit_score(sl, g, qb, b, qs, dst):
                            if sl is None:
                                nc.tensor.matmul(
                                    dst, kr[:, qb, g, :],
                                    qrot[0:64, 4 * g:4 * g + 4, qs],
                                    start=True, stop=True)
                                nc.vector.tensor_add(
                                    dst.rearrange("p (h q) -> p h q", h=4),
                                    dst.rearrange("p (h q) -> p h q", h=4),
                                    trilT[:, None, :]
                                    .to_broadcast([128, 4, 128]))
                            else:
                                X, s = sl
                                nc.tensor.matmul(
                                    dst, Kg[2 * X + b][0:66, s, g, :],
                                    qrot[0:66, 4 * g:4 * g + 4, qs],
                                    start=True, stop=True)

                        def emit_normalize(pvq, g, qb, qs):
                            # rec of denom row, broadcast via 1-row matmul,
                            # fused scale-copy into o_sb
                            rec = pb.tile([1, 4 * 128], f32r, tag="rec",
                                          name="rec")
                            with nc.allow_low_precision(
                                    reason="f32r recip for PV scale"):
                                nc.vector.reciprocal(rec[:], pvq[HD:HD + 1, :])
                            bc = Pbc.tile([64, 4 * 128], f32, tag="bcpo",
                                          name="bc")
                            nc.tensor.matmul(bc[:], ones128[0:1, 0:64], rec[:],
                                             start=True, stop=True)
                            bc_sb = pb.tile([64, 4 * 128], f32, tag="bcs",
                                            name="bc_sb")
                            nc.vector.tensor_copy(bc_sb[:], bc[:])
                            for hh in range(4):
                                h_ = 4 * g + hh
                                nc.vector.tensor_mul(
                                    o_sb[64 * (hh % 2):64 * (hh % 2) + 64,
                                         h_ // 2, qs],
                                    pvq[0:HD, hh * 128:(hh + 1) * 128],
                                    bc_sb[:, hh * 128:(hh + 1) * 128])

                        for qi in range(2):
                            seq = SEQ0 if qi == 0 else SEQ1
                            nslots = sum(len(p) for p in seq)
                            for b in range(2):
                                qb = 2 * qi + b
                                qs = slice(qb * 128, (qb + 1) * 128)
                                for gp in range(2):
                                    gs = (2 * gp, 2 * gp + 1)
                                    pvqs = {g: Ppv.tile([HD + 1, 4 * 128], f32,
                                                        tag="pvq",
                                                        name=f"pvq{g}")
                                            for g in gs}
                                    idx = 0
                                    for pair in seq:
                                        npair = len(pair)
                                        scs = {}
                                        for g in gs:
                                            sc = Psc.tile([128, 2, 4 * 128],
                                                          f32, tag="sc",
                                                          name="sc")
                                            scs[g] = sc
                                            for j, sl in enumerate(pair):
                                                emit_score(sl, g, qb, b, qs,
                                                           sc[:, j, :])
                                        prbs = {}
                                        for g in gs:
                                            probs = pb.tile(
                                                [128, 2, 4 * 128], bfl,
                                                tag="probs", bufs=6,
                                                name="probs")
                                            prbs[g] = probs
                                            nc.scalar.activation(
                                                probs[:, 0:npair, :],
                                                scs[g][:, 0:npair, :], AF.Exp)
                                        for g in gs:
                                            for j, sl in enumerate(pair):
                                                vsl = (v_s[:, qb, g, :]
                                                       if sl is None
                                                       else Vg[2 * sl[0] + b][
                                                           :, sl[1], g, :])
                                                nc.tensor.matmul(
                                                    pvqs[g][:], vsl,
                                                    prbs[g][:, j, :],
                                                    start=(idx + j == 0),
                                                    stop=(idx + j
                                                          == nslots - 1))
                                        idx += npair
                                    for g in gs:
                                        emit_normalize(pvqs[g], g, qb, qs)

                        # wo projection + residual (one 2MB weight load)
                        wo_t = pb.tile([128, KS, KS, 128], bfl, tag="wqo",
                                       name="wo_t")
                        nc.sync.dma_start(
                            wo_t[:],
                            wo_d[l * KS:(l + 1) * KS].rearrange(
                                "b p k m -> p b k m"))
                        for mb in range(KS):
                            po = Pbc.tile([128, TPC], f32, tag="bcpo",
                                          name="po")
                            for k in range(KS):
                                nc.tensor.matmul(po[:], wo_t[:, mb, k, :],
                                                 o_sb[:, k, :], start=(k == 0),
                                                 stop=(k == KS - 1))
                            nc.vector.tensor_add(hT[:, mb, :], hT[:, mb, :],
                                                 po[:])

                    # ======== phase C: mlp ===================================
                    with tc.tile_pool(name="phC", bufs=2) as pc_, \
                         tc.tile_pool(name="Pmlp", bufs=6, space="PSUM") as pml:
                        n2sb = pc_.tile([128, KS], f32, tag="nw", name="n2sb")
                        nc.sync.dma_start(n2sb[:], n2_d[l])
                        y = lp.tile([128, KS, TPC], bfl, tag="y", name="y")
                        rmsnorm(pml, pc_, hT, n2sb, y)
                        for qp in range(2):
                            m_sbs = []
                            for qh in range(2):
                                quarter = 2 * qp + qh
                                m_sb = pc_.tile([128, 8, TPC], bfl, tag="m",
                                                bufs=2, name=f"m_sb{qh}")
                                m_sbs.append(m_sb)
                                for mb4 in range(2):
                                    q8 = quarter * 2 + mb4
                                    w13_t = pc_.tile([128, 2, KS, 512], bfl,
                                                     tag="w13", bufs=3,
                                                     name="w13_t")
                                    nc.sync.dma_start(
                                        w13_t[:],
                                        w13_d[l * 8 + q8].rearrange(
                                            "w p k m -> p w k m"))
                                    for mbi in range(4):
                                        pu = pml.tile([128, TPC], f32,
                                                      tag="mm", name="pu")
                                        for k in range(KS):
                                            nc.tensor.matmul(
                                                pu[:],
                                                w13_t[:, 0, k,
                                                      mbi * 128:(mbi + 1) * 128],
                                                y[:, k, :], start=(k == 0),
                                                stop=(k == KS - 1))
                                        s_sb = pc_.tile([128, TPC], f32r,
                                                        tag="s", name="s_sb")
                                        nc.scalar.activation(s_sb[:], pu[:],
                                                             AF.Silu)
                                        pg = pml.tile([128, TPC], f32,
                                                      tag="mm", name="pg")
                                        for k in range(KS):
                                            nc.tensor.matmul(
                                                pg[:],
                                                w13_t[:, 1, k,
                                                      mbi * 128:(mbi + 1) * 128],
                                                y[:, k, :], start=(k == 0),
                                                stop=(k == KS - 1))
                                        nc.vector.tensor_mul(
                                            m_sb[:, mb4 * 4 + mbi, :],
                                            s_sb[:], pg[:])
                            # w2: one 512KB load per mb covering both quarters
                            # of the pair; 16-deep accumulation chain
                            for mb in range(KS):
                                w2_t = pc_.tile([128, 2, 8, 128], bfl,
                                                tag="w2", name="w2_t")
                                nc.sync.dma_start(
                                    w2_t[:], w2_d[(l * 2 + qp) * KS + mb])
                                pd = pml.tile([128, TPC], f32, tag="mm",
                                              name="pd")
                                for qh in range(2):
                                    for ks_ in range(8):
                                        nc.tensor.matmul(
                                            pd[:], w2_t[:, qh, ks_, :],
                                            m_sbs[qh][:, ks_, :],
                                            start=(qh == 0 and ks_ == 0),
                                            stop=(qh == 1 and ks_ == 7))
                                nc.vector.tensor_add(hT[:, mb, :],
                                                     hT[:, mb, :], pd[:])

            # ======== final norm + lm head ===================================
            with tc.tile_pool(name="phL", bufs=2) as pl_, \
                 tc.tile_pool(name="Plm", bufs=6, space="PSUM") as plp:
                fnsb = pl_.tile([128, KS], f32, tag="nw", name="fnsb")
                nc.sync.dma_start(fnsb[:], fnw_d[:])
                hn = pl_.tile([128, KS, TPC], bfl, tag="hn", bufs=1, name="hn")
                rmsnorm(plp, pl_, hT, fnsb, hn)
                for vc0 in range(0, [NVCH, 1][skip_lm], 2):
                    nv = min(2, NVCH - vc0)
                    emb_t = pl_.tile([128, 2, KS, 512], bfl, tag="emb",
                                     bufs=2, name="emb_t")
                    nc.sync.dma_start(
                        emb_t[:, 0:nv],
                        emb_d[vc0:vc0 + nv].rearrange("b p k m -> p b k m"))
                    for sub in range(nv):
                        vch = vc0 + sub
                        n = min(512, V - vch * 512)
                        ol = pl_.tile([128, 4, 512], bfl, tag="ol", bufs=2,
                                      name="ol")
                        for tb in range(4):
                            plm = plp.tile([128, 512], f32, tag="mm",
                                           name="plm")
                            for k in range(KS):
                                nc.tensor.matmul(
                                    plm[:, 0:n],
                                    hn[:, k, tb * 128:(tb + 1) * 128],
                                    emb_t[:, sub, k, 0:n], start=(k == 0),
                                    stop=(k == KS - 1))
                            # alternate ACT/DVE so the PSUM-evacuation
                            # copies aren't a single-engine stream
                            if tb % 2 == 0:
                                nc.scalar.copy(ol[:, tb, 0:n], plm[:, 0:n])
                            else:
                                nc.vector.tensor_copy(ol[:, tb, 0:n],
                                                      plm[:, 0:n])
                        nc.sync.dma_start(
                            out_d[:, vch * 512:vch * 512 + n].rearrange(
                                "(t p) v -> p t v", p=128),
                            ol[:, :, 0:n])
    nc.compile()
    _NC_CACHE = nc
    return nc


def host_prep(inputs):
    """Build per-core in_maps. Weights are pre-transposed host-side into the
    exact SBUF tile layouts (contiguous DMA runs) and cast to bf16."""
    ids = np.asarray(inputs['input_ids'])
    emb = np.asarray(inputs['tok_embed'], np.float32)
    wq = np.asarray(inputs['wq'], np.float32)
    wk = np.asarray(inputs['wk'], np.float32)
    wv = np.asarray(inputs['wv'], np.float32)
    wo = np.asarray(inputs['wo'], np.float32)
    n1 = np.asarray(inputs['norm1_w'], np.float32)
    n2 = np.asarray(inputs['norm2_w'], np.float32)
    w1 = np.asarray(inputs['w1'], np.float32)
    w2 = np.asarray(inputs['w2'], np.float32)
    w3 = np.asarray(inputs['w3'], np.float32)
    fnw = np.asarray(inputs['final_norm_w'], np.float32)

    cos, sin = rope_tables()
    scale = np.float32(HD ** -0.5)
    sgn = np.concatenate([-np.ones(HD // 2, np.float32),
                          np.ones(HD // 2, np.float32)])

    # weight layouts: target[l, mb, p, k, mm] = w[l, mb*128+mm, k*128+p]
    wqT = np.ascontiguousarray(
        wq.reshape(L, KS, 128, KS, 128).transpose(0, 1, 4, 3, 2)
    ).reshape(L * KS, 128, KS, 128).astype(bf16)
    woT = np.ascontiguousarray(
        wo.reshape(L, KS, 128, KS, 128).transpose(0, 1, 4, 3, 2)
    ).reshape(L * KS, 128, KS, 128).astype(bf16)
    # [l, p, k, m] = w[l, m, k*128+p], m in 0..255
    wkT = np.ascontiguousarray(
        wk.reshape(L, 256, KS, 128).transpose(0, 3, 2, 1)).astype(bf16)
    wvT = np.ascontiguousarray(
        wv.reshape(L, 256, KS, 128).transpose(0, 3, 2, 1)).astype(bf16)
    # [l, q8, w, p, k, mm(512)] = w{1,3}[l, q8*512+mm, k*128+p]
    w1T = np.ascontiguousarray(
        w1.reshape(L, 8, 512, KS, 128).transpose(0, 1, 4, 3, 2)
    ).reshape(L * 8, 128, KS, 512).astype(bf16)
    w3T = np.ascontiguousarray(
        w3.reshape(L, 8, 512, KS, 128).transpose(0, 1, 4, 3, 2)
    ).reshape(L * 8, 128, KS, 512).astype(bf16)
    w13T = np.ascontiguousarray(
        np.stack([w1T, w3T], axis=1))
    # [l, qp, mb, p, qh, ks, mm] = w2[l, mb*128+mm, (2qp+qh)*1024+ks*128+p]
    w2T = np.ascontiguousarray(
        w2.reshape(L, KS, 128, 2, 2, 8, 128).transpose(0, 3, 1, 6, 4, 5, 2)
    ).reshape(L * 2 * KS, 128, 2, 8, 128).astype(bf16)
    # [vch, p, k, vv] = emb[vch*512+vv, k*128+p]
    embp = np.zeros((NVCH * 512, D), np.float32)
    embp[0:V] = emb
    embT = np.ascontiguousarray(
        embp.reshape(NVCH, 512, KS, 128).transpose(0, 3, 2, 1)).astype(bf16)

    shared = {
        "wqT": wqT, "woT": woT, "wkT": wkT, "wvT": wvT,
        "w13T": w13T, "w2T": w2T, "embT": embT,
        "n1": np.ascontiguousarray(n1.reshape(L, KS, 128).transpose(0, 2, 1)),
        "n2": np.ascontiguousarray(n2.reshape(L, KS, 128).transpose(0, 2, 1)),
        "fnw": np.ascontiguousarray(fnw.reshape(KS, 128).T),
        "p64": np.eye(HD, dtype=np.float32)[
            np.concatenate([np.arange(32, 64), np.arange(0, 32)])].T.copy(),
        "ones128": np.ones((128, 128), np.float32),
        # [tk, tq] orientation: invalid where tk > tq
        "trilT": np.tril(np.full((128, 128), NEG, np.float32), -1),
    }
    # token block order: [b0 low-chunk, b1 low, b0 high, b1 high] so block
    # qb = 2*qi + b. qflag row qi selects that qi's 256-token range.
    qf = np.zeros((2, NH, TPC), np.float32)
    for qi in range(2):
        qf[qi, :, qi * 256:(qi + 1) * 256] = 1.0
    shared["qflag"] = qf.astype(bf16)

    in_maps = []
    for c in range(NC):
        pos = []
        for j in core_chunks(c):
            for b in range(B):
                pos.extend((b, j * CH + i) for i in range(CH))
        bidx = np.array([p[0] for p in pos])
        pidx = np.array([p[1] for p in pos])
        x0 = emb[ids[bidx, pidx]]                    # [512, D]
        # x0T[p, k, t] = x0[t, k*128+p]
        x0T = np.ascontiguousarray(
            x0.reshape(TPC, KS, 128).transpose(2, 1, 0))
        cq = np.ascontiguousarray(cos[pidx].T) * scale
        sq = np.ascontiguousarray(sin[pidx].T) * sgn[:, None] * scale
        ck = np.ascontiguousarray(cos[pidx].T)
        sk = np.ascontiguousarray(sin[pidx].T) * sgn[:, None]
        # kbias [half, qi, slot, g, m]: half 0 slot s = chunk s, half 1
        # slot s = chunk 15-s; window chunk valid iff chunk < own chunk j
        kb = np.zeros((2, 2, 8, KVH, 128), np.float32)
        for qi, j in enumerate(core_chunks(c)):
            for X in range(2):
                for s in range(8):
                    ch = s if X == 0 else NCH - 1 - s
                    kb[X, qi, s] = 0.0 if ch < j else NEG
        m = {"x0T": x0T, "cosq": cq.astype(np.float32),
             "sinq": sq.astype(np.float32), "cosk": ck.astype(np.float32),
             "sink": sk.astype(np.float32),
             "kbias": kb.astype(bf16)}
        m.update(shared)
        in_maps.append(m)
    return in_maps


def unshard(results):
    out = np.zeros((B, S, V), np.float32)
    for c in range(NC):
        logits = np.asarray(results[c]["out"], np.float32)
        for qi, j in enumerate(core_chunks(c)):
            for b in range(B):
                qb = 2 * qi + b
                out[b, j * CH:(j + 1) * CH] = logits[qb * 128:(qb + 1) * 128]
    return out


def kernel(**inputs) -> np.ndarray:
    from concourse.bass_utils import run_bass_kernel_spmd
    nc = build_nc()
    in_maps = host_prep(inputs)
    res = run_bass_kernel_spmd(nc, in_maps, core_ids=list(range(NC)),
                               trace=False)
    return unshard(res.results)



# revision 61
# speedup vs baseline: 1.0099x; 1.0099x over previous
"""Trainium2 Bass kernel for an 8-layer dense transformer (CloudTransformerMM).

Strategy: data-parallel over tokens (zigzag chunk pairing: core c owns chunks
{c, 15-c} of each batch) across 8 cores, per-layer K/V AllGather.
Feature-major residual stream [D, tokens] per core so projections need no
activation transposes.

v2: bf16 pre-transposed weights, k-major attention (K^T.Q scores, ones-column
softmax denominators, post-PV broadcast normalize), bf16 gathers.

v3+ changes vs v2:
- K+V packed into one AllGather per 128-token block (4/layer). Token order is
  [b0 low-chunk, b1 low, b0 high, b1 high], so each gather lands just before
  its consumers: qi=0 windows need only the low-chunk gathers, and qi=1's
  high-chunk slots run last (v2 stalled ~107us/layer on gather latency).
- Gather payloads laid out so Kg/Vg SBUF loads are large DMAs with >=520B
  contiguous runs (v2: 32 small strided DMAs per layer).
- Attention exp in slot pairs ([128,1024] per ACT op) to amortize the ~300ns
  fixed ACT cost; own-chunk slot unpaired so it needs no gather. Sibling
  (g even, g odd) head-groups are interleaved pair-step by pair-step so PE
  and ACT always hold independent work while MM->exp->PV dependencies resolve.
- rmsnorm: mean+eps folded into ACT sqrt's affine pre-scale, approximate DVE
  reciprocal (~51 ULP), squares alternated ACT/DVE.
- Coalesced weight streams (wq/wo 1x2MB per layer, w1+w3 2MB pairs, w2 512KB
  pairs with 16-deep accumulation chains, lm-head embeddings 2MB pairs,
  single merged logits store per vocab chunk) - per-DMA fixed cost through
  the tunnel-measured ~1-2us made 1029 DMAs/run a real tax (now ~505).
- bf16 logits output (halves the store; rel-err impact ~0.2% in quadrature).
Precision: bf16 weights x f32r activations on PE, bf16 score path, fp32
PSUM + residual.

Measured (8-core TRN2, steady-state pipelined-dispatch slope): 4.74 ms at
round 2; TimelineSim cost model 3.85 ms for the current build.
"""
import math
import sys

sys.path.insert(0, '/opt/trn_rl_repo')

import numpy as np
import ml_dtypes

B, S, D = 2, 2048, 1024
NH, KVH, HD = 16, 4, 64
L, DFF, V = 8, 4096, 32000
THETA, YSCALE, YALPHA, YBETA = 10000.0, 40.0, 1.0, 32.0
ROPE_MAX = 2048 * 40
EPS = 1e-6

NC = 8
NCH = 16
CH = S // NCH          # 128
TPC = 2 * 2 * CH       # 512
KS = D // 128          # 8
NEG = -1.0e30
NVCH = (V + 511) // 512  # 63 (last chunk zero-padded host-side)
KPAY = 64 * 4 * 128      # K payload elems per (b, half) in the kv gather
VPAY = 128 * 4 * 65      # V payload elems (includes the ones column)
PAY = KPAY + VPAY

bf16 = ml_dtypes.bfloat16


def rope_tables():
    inv_freq = 1.0 / THETA ** (np.arange(0, HD, 2, dtype=np.float32) / HD)
    wavelengths = 2.0 * math.pi / inv_freq
    r = ROPE_MAX / wavelengths
    gamma = np.clip((r - YALPHA) / (YBETA - YALPHA), 0.0, 1.0)
    inv_freq = inv_freq * ((1.0 - gamma) / YSCALE + gamma)
    t = np.arange(S, dtype=np.float32)
    freqs = np.outer(t, inv_freq)
    emb = np.concatenate([freqs, freqs], axis=-1)
    emb = emb / math.sqrt(0.1 * math.log(YSCALE) + 1.0)
    return np.cos(emb).astype(np.float32), np.sin(emb).astype(np.float32)


def core_chunks(c):
    return [c, NCH - 1 - c]


def chunk_owner(lk):
    oc = min(lk, NCH - 1 - lk)
    return oc, (0 if lk == oc else 1)


_NC_CACHE = None


def build_nc(no_coll=False, skip_layers=False, skip_lm=False):
    """no_coll / skip_layers / skip_lm build timing-shape variants for
    performance attribution experiments only, never for real output."""
    global _NC_CACHE
    variant = no_coll or skip_layers or skip_lm
    if _NC_CACHE is not None and not variant:
        return _NC_CACHE
    import concourse.mybir as mybir
    import concourse.tile as tile
    from concourse import bacc

    f32 = mybir.dt.float32
    f32r = mybir.dt.float32r
    bfl = mybir.dt.bfloat16
    AF = mybir.ActivationFunctionType
    ALU = mybir.AluOpType

    nc = bacc.Bacc("TRN2", target_bir_lowering=False, debug=False,
                   enable_asserts=True, num_devices=NC)

    def din(name, shape, dt):
        return nc.dram_tensor(name, shape, dt, kind="ExternalInput").ap()

    x0T_d = din("x0T", [128, KS, TPC], f32)
    wq_d = din("wqT", [L * KS, 128, KS, 128], bfl)     # [l*8+mb]
    wk_d = din("wkT", [L, 128, KS, KVH * HD], bfl)
    wv_d = din("wvT", [L, 128, KS, KVH * HD], bfl)
    wo_d = din("woT", [L * KS, 128, KS, 128], bfl)     # [l*8+mb]
    w13_d = din("w13T", [L * 8, 2, 128, KS, 512], bfl)  # [l*8+q8][w1|w3]
    w2_d = din("w2T", [L * 2 * KS, 128, 2, 8, 128], bfl)  # [(l*2+qp)*8+mb]
    n1_d = din("n1", [L, 128, KS], f32)
    n2_d = din("n2", [L, 128, KS], f32)
    fnw_d = din("fnw", [128, KS], f32)
    emb_d = din("embT", [NVCH, 128, KS, 512], bfl)
    cosq_d = din("cosq", [HD, TPC], f32)
    sinq_d = din("sinq", [HD, TPC], f32)
    cosk_d = din("cosk", [HD, TPC], f32)
    sink_d = din("sink", [HD, TPC], f32)
    p64_d = din("p64", [HD, HD], f32r)
    trilT_d = din("trilT", [128, 128], f32)
    ones_d = din("ones128", [128, 128], f32r)
    kbias_d = din("kbias", [2, 2, 8, KVH, 128], bfl)  # [half, qi, slot, g, m]
    qflag_d = din("qflag", [2, NH, TPC], bfl)
    out_d = nc.dram_tensor("out", [TPC, V], bfl, kind="ExternalOutput").ap()

    with tile.TileContext(nc) as tc, \
         tc.tile_pool(name="pers", bufs=1) as pers:
        hT = pers.tile([128, KS, TPC], f32, tag="hT", name="hT")
        qrot = pers.tile([128, NH, TPC], bfl, tag="qrot", name="qrot")
        cosq = pers.tile([HD, TPC], f32, tag="cosq", name="cosq")
        sinq = pers.tile([HD, TPC], f32, tag="sinq", name="sinq")
        cosk = pers.tile([HD, TPC], f32, tag="cosk", name="cosk")
        sink = pers.tile([HD, TPC], f32, tag="sink", name="sink")
        p64 = pers.tile([HD, HD], f32r, tag="p64", name="p64")
        trilT = pers.tile([128, 128], f32, tag="trilT", name="trilT")
        ones128 = pers.tile([128, 128], f32r, tag="ones128", name="ones128")
        eps128 = pers.tile([128, 1], f32, tag="eps128", name="eps128")
        nc.vector.memset(eps128[:], float(EPS))
        # persistent gathered-K/V tiles: kbias rows (64:66) are layer-
        # invariant, loaded once; only rows 0:64 / V data stream per layer
        Kg = [pers.tile([66, 8, KVH, 128], bfl, tag=f"Kg{q}", name=f"Kg{q}")
              for q in range(4)]
        Vg = [pers.tile([128, 8, KVH, HD + 1], bfl, tag=f"Vg{q}",
                        name=f"Vg{q}") for q in range(4)]
        for q in range(4):
            nc.sync.dma_start(Kg[q][64:66], kbias_d[q // 2])

        nc.sync.dma_start(hT[:], x0T_d[:])
        nc.sync.dma_start(qrot[64:66, :, :], qflag_d[:])
        for t_, d_ in ((cosq, cosq_d), (sinq, sinq_d), (cosk, cosk_d),
                       (sink, sink_d), (p64, p64_d), (trilT, trilT_d),
                       (ones128, ones_d)):
            nc.sync.dma_start(t_[:], d_[:])

        def rmsnorm(P, smp, src, w_sb, dst):
            # inv = rsqrt(mean(x^2)+eps): ACT sqrt (with mean+eps folded into
            # its affine pre-scale) + fast approximate DVE reciprocal
            # (~51 ULP, ~5x faster than the bit-exact iterative divide).
            ssp = P.tile([128, TPC], f32, tag="mm", name="ssp")
            for sub in range(KS):
                sq = smp.tile([128, TPC], f32r, tag="sq", name="sq")
                # alternate ACT/DVE so the square stream isn't single-engine
                if sub % 2 == 0:
                    nc.scalar.activation(sq[:], src[:, sub, :], AF.Square)
                else:
                    nc.vector.tensor_mul(sq[:], src[:, sub, :], src[:, sub, :])
                nc.tensor.matmul(ssp[:], ones128[:], sq[:],
                                 start=(sub == 0), stop=(sub == KS - 1))
            sd = smp.tile([128, TPC], f32, tag="sd", name="sd")
            nc.scalar.activation(sd[:], ssp[:], AF.Sqrt, eps128[:], 1.0 / D)
            inv = smp.tile([128, TPC], f32, tag="inv", name="inv")
            nc.vector.reciprocal_approx_fast(inv[:], sd[:])
            for sub in range(KS):
                nc.vector.scalar_tensor_tensor(
                    dst[:, sub, :], src[:, sub, :], w_sb[:, sub:sub + 1],
                    inv[:], ALU.mult, ALU.mult)

        with tc.tile_pool(name="dram", bufs=2, space="DRAM") as dram:

            for l in range([0, L][not skip_layers]):
                with tc.tile_pool(name="layerp", bufs=1) as lp:
                    xn = lp.tile([128, KS, TPC], bfl, tag="xn", name="xn")
                    # block-major: [p, block, g, m] so gather packs read 1KB runs
                    kr = lp.tile([64, 4, KVH, 128], bfl, tag="kr", name="kr")
                    v_s = lp.tile([128, 4, KVH, HD + 1], bfl, tag="v_s",
                                  name="v_s")
                    o_sb = lp.tile([128, KS, TPC], bfl, tag="o", name="o_sb")

                    # ======== phase A: norm1, k/v/q proj + rope + gathers ====
                    kv_out = []
                    with tc.tile_pool(name="phA", bufs=2) as pa, \
                         tc.tile_pool(name="PpsA", bufs=4,
                                      space="PSUM") as ppa:
                        n1sb = pa.tile([128, KS], f32, tag="nw", name="n1sb")
                        nc.sync.dma_start(n1sb[:], n1_d[l])
                        rmsnorm(ppa, pa, hT, n1sb, xn)

                        # k projection + rope (first, to hide gather latency)
                        wk_t = pa.tile([128, KS, KVH * HD], bfl, tag="wkv",
                                       name="wk_t")
                        nc.sync.dma_start(wk_t[:], wk_d[l])
                        k_s = pa.tile([64, KVH, TPC], f32r, tag="k_s", bufs=1,
                                      name="k_s")
                        for mb in range(2):
                            pk = ppa.tile([128, TPC], f32, tag="mm", name="pk")
                            for k in range(KS):
                                nc.tensor.matmul(
                                    pk[:], wk_t[:, k, mb * 128:(mb + 1) * 128],
                                    xn[:, k, :], start=(k == 0),
                                    stop=(k == KS - 1))
                            nc.scalar.copy(k_s[:, 2 * mb, :], pk[0:64, :])
                            nc.scalar.copy(k_s[:, 2 * mb + 1, :], pk[64:128, :])
                        for g in range(KVH):
                            psh = ppa.tile([64, TPC], f32, tag="mm", name="psh")
                            nc.tensor.matmul(psh[:], p64[:], k_s[:, g, :],
                                             start=True, stop=True)
                            tA = pa.tile([64, TPC], f32, tag="tA", name="tA")
                            nc.vector.tensor_mul(tA[:], psh[:], sink[:])
                            tB = pa.tile([64, TPC], f32, tag="tB", name="tB")
                            nc.vector.tensor_mul(tB[:], k_s[:, g, :], cosk[:])
                            nc.vector.tensor_add(
                                kr[:, :, g, :],
                                tA[:].rearrange("p (q m) -> p q m", q=4),
                                tB[:].rearrange("p (q m) -> p q m", q=4))

                        # v projection (token-major, bf16, +ones col)
                        wv_t = pa.tile([128, KS, KVH * HD], bfl, tag="wkv",
                                       name="wv_t")
                        nc.sync.dma_start(wv_t[:], wv_d[l])
                        nc.vector.memset(v_s[:, :, :, HD:HD + 1], 1.0)
                        # combined K+V gathers, one per 128-token block,
                        # each issued as soon as its block's V is ready.
                        # Token order is [b0 low, b1 low, b0 high, b1 high],
                        # so quarter q carries (half X=q//2, batch b=q%2) and
                        # lands just before its consumers need it.
                        for tb in range(4):
                            pv_ = ppa.tile([128, KVH * HD], f32, tag="mm",
                                         name="pv_")
                            for k in range(KS):
                                nc.tensor.matmul(
                                    pv_[:], xn[:, k, tb * 128:(tb + 1) * 128],
                                    wv_t[:, k, :], start=(k == 0),
                                    stop=(k == KS - 1))
                            nc.scalar.copy(
                                v_s[:, tb, :, 0:HD],
                                pv_[:].rearrange("p (g h) -> p g h", g=KVH))
                            kv_in = dram.tile([PAY], bfl, tag="kv_i", bufs=4,
                                              name=f"kv_in{tb}")
                            nc.sync.dma_start(
                                kv_in[0:KPAY].rearrange(
                                    "(p g m) -> p g m", p=64, g=KVH),
                                kr[:, tb])
                            nc.sync.dma_start(
                                kv_in[KPAY:PAY].rearrange(
                                    "(p g h) -> p g h", p=128, g=KVH),
                                v_s[:, tb, :, :])
                            ko = dram.tile([NC, PAY], bfl, tag="kv_o", bufs=4,
                                           addr_space="Shared",
                                           name=f"kv_out{tb}")
                            if not no_coll:
                                nc.gpsimd.collective_compute(
                                    "AllGather", ALU.bypass,
                                    replica_groups=[list(range(NC))],
                                    ins=[kv_in.opt()], outs=[ko.opt()])
                            kv_out.append(ko)

                        # q projection + rope (one 2MB weight load)
                        wq_t = pa.tile([128, KS, KS, 128], bfl, tag="wqo",
                                       name="wq_t")
                        nc.sync.dma_start(
                            wq_t[:],
                            wq_d[l * KS:(l + 1) * KS].rearrange(
                                "b p k m -> p b k m"))
                        for mb in range(KS):
                            pq = ppa.tile([128, TPC], f32, tag="mm", name="pq")
                            for k in range(KS):
                                nc.tensor.matmul(pq[:], wq_t[:, mb, k, :],
                                                 xn[:, k, :], start=(k == 0),
                                                 stop=(k == KS - 1))
                            q_s = pa.tile([64, 2, TPC], f32r, tag="q_s",
                                          name="q_s")
                            nc.scalar.copy(q_s[:, 0, :], pq[0:64, :])
                            nc.scalar.copy(q_s[:, 1, :], pq[64:128, :])
                            for hh in range(2):
                                h_ = 2 * mb + hh
                                psh = ppa.tile([64, TPC], f32, tag="mm",
                                             name="pshq")
                                nc.tensor.matmul(psh[:], p64[:], q_s[:, hh, :],
                                                 start=True, stop=True)
                                tA = pa.tile([64, TPC], f32, tag="tA",
                                             name="tAq")
                                nc.vector.tensor_mul(tA[:], psh[:], sinq[:])
                                tB = pa.tile([64, TPC], f32, tag="tB",
                                             name="tBq")
                                nc.vector.tensor_mul(tB[:], q_s[:, hh, :],
                                                     cosq[:])
                                nc.vector.tensor_add(qrot[0:64, h_, :],
                                                     tA[:], tB[:])

                    # ======== phase B: attention (k-major, A/B halves) =======
                    # Gather half 0 delivers chunks 0..7 (slot s = chunk s),
                    # half 1 delivers chunks 15..8 (slot s = chunk 15-s).
                    # qi=0 windows need only half 0; qi=1 needs both, and its
                    # half-1 slots run last so gather 1 latency hides behind
                    # the whole qi=0 pass.
                    with tc.tile_pool(name="phB", bufs=2) as pb, \
                         tc.tile_pool(name="Psc", bufs=2, space="PSUM") as Psc, \
                         tc.tile_pool(name="Ppv", bufs=2, space="PSUM") as Ppv, \
                         tc.tile_pool(name="Pbc", bufs=2, space="PSUM") as Pbc:
                        for q in range(4):
                            nc.sync.dma_start(
                                Kg[q][0:64],
                                kv_out[q][:, 0:KPAY].rearrange(
                                    "c (p g m) -> p c g m", p=64, g=KVH))
                            nc.sync.dma_start(
                                Vg[q][:],
                                kv_out[q][:, KPAY:PAY].rearrange(
                                    "c (p g h) -> p c g h", p=128, g=KVH))

                        # slot sequences: None = own chunk (local kr/v_s);
                        # (X, s) = gathered half X slot s. Window slots are
                        # exp'd in pairs to amortize ACT fixed cost.
                        SEQ0 = [[None], [(0, 0), (0, 1)], [(0, 2), (0, 3)],
                                [(0, 4), (0, 5)], [(0, 6)]]
                        SEQ1 = [[None], [(0, 0), (0, 1)], [(0, 2), (0, 3)],
                                [(0, 4), (0, 5)], [(0, 6), (0, 7)],
                                [(1, 1), (1, 2)], [(1, 3), (1, 4)],
                                [(1, 5), (1, 6)], [(1, 7)]]
                        # groups (qi, b, g) are processed in sibling pairs
                        # (g even, g odd) with their score/exp/PV pair-steps
                        # interleaved, so PE and ACT always have the other
                        # group's independent work while a cross-engine
                        # dependency (MM -> exp -> PV) resolves.
                        def emit_score(sl, g, qb, b, qs, dst):
                            if sl is None:
                                nc.tensor.matmul(
                                    dst, kr[:, qb, g, :],
                                    qrot[0:64, 4 * g:4 * g + 4, qs],
                                    start=True, stop=True)
                                nc.vector.tensor_add(
                                    dst.rearrange("p (h q) -> p h q", h=4),
                                    dst.rearrange("p (h q) -> p h q", h=4),
                                    trilT[:, None, :]
                                    .to_broadcast([128, 4, 128]))
                            else:
                                X, s = sl
                                nc.tensor.matmul(
                                    dst, Kg[2 * X + b][0:66, s, g, :],
                                    qrot[0:66, 4 * g:4 * g + 4, qs],
                                    start=True, stop=True)

                        def emit_normalize(pvq, g, qb, qs):
                            # rec of denom row, broadcast via 1-row matmul,
                            # fused scale-copy into o_sb
                            rec = pb.tile([1, 4 * 128], f32r, tag="rec",
                                          name="rec")
                            with nc.allow_low_precision(
                                    reason="f32r recip for PV scale"):
                                nc.vector.reciprocal(rec[:], pvq[HD:HD + 1, :])
                            bc = Pbc.tile([64, 4 * 128], f32, tag="bcpo",
                                          name="bc")
                            nc.tensor.matmul(bc[:], ones128[0:1, 0:64], rec[:],
                                             start=True, stop=True)
                            bc_sb = pb.tile([64, 4 * 128], f32, tag="bcs",
                                            name="bc_sb")
                            nc.vector.tensor_copy(bc_sb[:], bc[:])
                            for hh in range(4):
                                h_ = 4 * g + hh
                                nc.vector.tensor_mul(
                                    o_sb[64 * (hh % 2):64 * (hh % 2) + 64,
                                         h_ // 2, qs],
                                    pvq[0:HD, hh * 128:(hh + 1) * 128],
                                    bc_sb[:, hh * 128:(hh + 1) * 128])

                        for qi in range(2):
                            seq = SEQ0 if qi == 0 else SEQ1
                            nslots = sum(len(p) for p in seq)
                            for b in range(2):
                                qb = 2 * qi + b
                                qs = slice(qb * 128, (qb + 1) * 128)
                                for gp in range(2):
                                    gs = (2 * gp, 2 * gp + 1)
                                    pvqs = {g: Ppv.tile([HD + 1, 4 * 128], f32,
                                                        tag="pvq",
                                                        name=f"pvq{g}")
                                            for g in gs}
                                    idx = 0
                                    for pair in seq:
                                        npair = len(pair)
                                        scs = {}
                                        for g in gs:
                                            sc = Psc.tile([128, 2, 4 * 128],
                                                          f32, tag="sc",
                                                          name="sc")
                                            scs[g] = sc
                                            for j, sl in enumerate(pair):
                                                emit_score(sl, g, qb, b, qs,
                                                           sc[:, j, :])
                                        prbs = {}
                                        for g in gs:
                                            probs = pb.tile(
                                                [128, 2, 4 * 128], bfl,
                                                tag="probs", bufs=6,
                                                name="probs")
                                            prbs[g] = probs
                                            nc.scalar.activation(
                                                probs[:, 0:npair, :],
                                                scs[g][:, 0:npair, :], AF.Exp)
                                        for g in gs:
                                            for j, sl in enumerate(pair):
                                                vsl = (v_s[:, qb, g, :]
                                                       if sl is None
                                                       else Vg[2 * sl[0] + b][
                                                           :, sl[1], g, :])
                                                nc.tensor.matmul(
                                                    pvqs[g][:], vsl,
                                                    prbs[g][:, j, :],
                                                    start=(idx + j == 0),
                                                    stop=(idx + j
                                                          == nslots - 1))
                                        idx += npair
                                    for g in gs:
                                        emit_normalize(pvqs[g], g, qb, qs)

                        # wo projection + residual (one 2MB weight load)
                        wo_t = pb.tile([128, KS, KS, 128], bfl, tag="wqo",
                                       name="wo_t")
                        nc.sync.dma_start(
                            wo_t[:],
                            wo_d[l * KS:(l + 1) * KS].rearrange(
                                "b p k m -> p b k m"))
                        for mb in range(KS):
                            po = Pbc.tile([128, TPC], f32, tag="bcpo",
                                          name="po")
                            for k in range(KS):
                                nc.tensor.matmul(po[:], wo_t[:, mb, k, :],
                                                 o_sb[:, k, :], start=(k == 0),
                                                 stop=(k == KS - 1))
                            nc.vector.tensor_add(hT[:, mb, :], hT[:, mb, :],
                                                 po[:])

                    # ======== phase C: mlp ===================================
                    with tc.tile_pool(name="phC", bufs=2) as pc_, \
                         tc.tile_pool(name="Pmlp", bufs=6, space="PSUM") as pml:
                        n2sb = pc_.tile([128, KS], f32, tag="nw", name="n2sb")
                        nc.sync.dma_start(n2sb[:], n2_d[l])
                        y = lp.tile([128, KS, TPC], bfl, tag="y", name="y")
                        rmsnorm(pml, pc_, hT, n2sb, y)
                        for qp in range(2):
                            m_sbs = []
                            for qh in range(2):
                                quarter = 2 * qp + qh
                                m_sb = pc_.tile([128, 8, TPC], bfl, tag="m",
                                                bufs=2, name=f"m_sb{qh}")
                                m_sbs.append(m_sb)
                                for mb4 in range(2):
                                    q8 = quarter * 2 + mb4
                                    w13_t = pc_.tile([128, 2, KS, 512], bfl,
                                                     tag="w13", bufs=3,
                                                     name="w13_t")
                                    nc.sync.dma_start(
                                        w13_t[:],
                                        w13_d[l * 8 + q8].rearrange(
                                            "w p k m -> p w k m"))
                                    for mbi in range(4):
                                        pu = pml.tile([128, TPC], f32,
                                                      tag="mm", name="pu")
                                        for k in range(KS):
                                            nc.tensor.matmul(
                                                pu[:],
                                                w13_t[:, 0, k,
                                                      mbi * 128:(mbi + 1) * 128],
                                                y[:, k, :], start=(k == 0),
                                                stop=(k == KS - 1))
                                        s_sb = pc_.tile([128, TPC], f32r,
                                                        tag="s", name="s_sb")
                                        nc.scalar.activation(s_sb[:], pu[:],
                                                             AF.Silu)
                                        pg = pml.tile([128, TPC], f32,
                                                      tag="mm", name="pg")
                                        for k in range(KS):
                                            nc.tensor.matmul(
                                                pg[:],
                                                w13_t[:, 1, k,
                                                      mbi * 128:(mbi + 1) * 128],
                                                y[:, k, :], start=(k == 0),
                                                stop=(k == KS - 1))
                                        nc.vector.tensor_mul(
                                            m_sb[:, mb4 * 4 + mbi, :],
                                            s_sb[:], pg[:])
                            # w2: one 512KB load per mb covering both quarters
                            # of the pair; 16-deep accumulation chain
                            for mb in range(KS):
                                w2_t = pc_.tile([128, 2, 8, 128], bfl,
                                                tag="w2", name="w2_t")
                                nc.sync.dma_start(
                                    w2_t[:], w2_d[(l * 2 + qp) * KS + mb])
                                pd = pml.tile([128, TPC], f32, tag="mm",
                                              name="pd")
                                for qh in range(2):
                                    for ks_ in range(8):
                                        nc.tensor.matmul(
                                            pd[:], w2_t[:, qh, ks_, :],
                                            m_sbs[qh][:, ks_, :],
                                            start=(qh == 0 and ks_ == 0),
                                            stop=(qh == 1 and ks_ == 7))
                                nc.vector.tensor_add(hT[:, mb, :],
                                                     hT[:, mb, :], pd[:])

            # ======== final norm + lm head ===================================
            with tc.tile_pool(name="phL", bufs=2) as pl_, \
                 tc.tile_pool(name="Plm", bufs=6, space="PSUM") as plp:
                fnsb = pl_.tile([128, KS], f32, tag="nw", name="fnsb")
                nc.sync.dma_start(fnsb[:], fnw_d[:])
                hn = pl_.tile([128, KS, TPC], bfl, tag="hn", bufs=1, name="hn")
                rmsnorm(plp, pl_, hT, fnsb, hn)
                for vc0 in range(0, [NVCH, 1][skip_lm], 2):
                    nv = min(2, NVCH - vc0)
                    emb_t = pl_.tile([128, 2, KS, 512], bfl, tag="emb",
                                     bufs=2, name="emb_t")
                    nc.sync.dma_start(
                        emb_t[:, 0:nv],
                        emb_d[vc0:vc0 + nv].rearrange("b p k m -> p b k m"))
                    for sub in range(nv):
                        vch = vc0 + sub
                        n = min(512, V - vch * 512)
                        ol = pl_.tile([128, 4, 512], bfl, tag="ol", bufs=2,
                                      name="ol")
                        for tb in range(4):
                            plm = plp.tile([128, 512], f32, tag="mm",
                                           name="plm")
                            for k in range(KS):
                                nc.tensor.matmul(
                                    plm[:, 0:n],
                                    hn[:, k, tb * 128:(tb + 1) * 128],
                                    emb_t[:, sub, k, 0:n], start=(k == 0),
                                    stop=(k == KS - 1))
                            # alternate ACT/DVE so the PSUM-evacuation
                            # copies aren't a single-engine stream
                            if tb % 2 == 0:
                                nc.scalar.copy(ol[:, tb, 0:n], plm[:, 0:n])
                            else:
                                nc.vector.tensor_copy(ol[:, tb, 0:n],
                                                      plm[:, 0:n])
                        nc.sync.dma_start(
                            out_d[:, vch * 512:vch * 512 + n].rearrange(
                                "(t p) v -> p t v", p=128),
                            ol[:, :, 0:n])
    nc.compile()
    _NC_CACHE = nc
    return nc


def host_prep(inputs):
    """Build per-core in_maps. Weights are pre-transposed host-side into the
    exact SBUF tile layouts (contiguous DMA runs) and cast to bf16."""
    ids = np.asarray(inputs['input_ids'])
    emb = np.asarray(inputs['tok_embed'], np.float32)
    wq = np.asarray(inputs['wq'], np.float32)
    wk = np.asarray(inputs['wk'], np.float32)
    wv = np.asarray(inputs['wv'], np.float32)
    wo = np.asarray(inputs['wo'], np.float32)
    n1 = np.asarray(inputs['norm1_w'], np.float32)
    n2 = np.asarray(inputs['norm2_w'], np.float32)
    w1 = np.asarray(inputs['w1'], np.float32)
    w2 = np.asarray(inputs['w2'], np.float32)
    w3 = np.asarray(inputs['w3'], np.float32)
    fnw = np.asarray(inputs['final_norm_w'], np.float32)

    cos, sin = rope_tables()
    scale = np.float32(HD ** -0.5)
    sgn = np.concatenate([-np.ones(HD // 2, np.float32),
                          np.ones(HD // 2, np.float32)])

    # weight layouts: target[l, mb, p, k, mm] = w[l, mb*128+mm, k*128+p]
    wqT = np.ascontiguousarray(
        wq.reshape(L, KS, 128, KS, 128).transpose(0, 1, 4, 3, 2)
    ).reshape(L * KS, 128, KS, 128).astype(bf16)
    woT = np.ascontiguousarray(
        wo.reshape(L, KS, 128, KS, 128).transpose(0, 1, 4, 3, 2)
    ).reshape(L * KS, 128, KS, 128).astype(bf16)
    # [l, p, k, m] = w[l, m, k*128+p], m in 0..255
    wkT = np.ascontiguousarray(
        wk.reshape(L, 256, KS, 128).transpose(0, 3, 2, 1)).astype(bf16)
    wvT = np.ascontiguousarray(
        wv.reshape(L, 256, KS, 128).transpose(0, 3, 2, 1)).astype(bf16)
    # [l, q8, w, p, k, mm(512)] = w{1,3}[l, q8*512+mm, k*128+p]
    w1T = np.ascontiguousarray(
        w1.reshape(L, 8, 512, KS, 128).transpose(0, 1, 4, 3, 2)
    ).reshape(L * 8, 128, KS, 512).astype(bf16)
    w3T = np.ascontiguousarray(
        w3.reshape(L, 8, 512, KS, 128).transpose(0, 1, 4, 3, 2)
    ).reshape(L * 8, 128, KS, 512).astype(bf16)
    w13T = np.ascontiguousarray(
        np.stack([w1T, w3T], axis=1))
    # [l, qp, mb, p, qh, ks, mm] = w2[l, mb*128+mm, (2qp+qh)*1024+ks*128+p]
    w2T = np.ascontiguousarray(
        w2.reshape(L, KS, 128, 2, 2, 8, 128).transpose(0, 3, 1, 6, 4, 5, 2)
    ).reshape(L * 2 * KS, 128, 2, 8, 128).astype(bf16)
    # [vch, p, k, vv] = emb[vch*512+vv, k*128+p]
    embp = np.zeros((NVCH * 512, D), np.float32)
    embp[0:V] = emb
    embT = np.ascontiguousarray(
        embp.reshape(NVCH, 512, KS, 128).transpose(0, 3, 2, 1)).astype(bf16)

    shared = {
        "wqT": wqT, "woT": woT, "wkT": wkT, "wvT": wvT,
        "w13T": w13T, "w2T": w2T, "embT": embT,
        "n1": np.ascontiguousarray(n1.reshape(L, KS, 128).transpose(0, 2, 1)),
        "n2": np.ascontiguousarray(n2.reshape(L, KS, 128).transpose(0, 2, 1)),
        "fnw": np.ascontiguousarray(fnw.reshape(KS, 128).T),
        "p64": np.eye(HD, dtype=np.float32)[
            np.concatenate([np.arange(32, 64), np.arange(0, 32)])].T.copy(),
        "ones128": np.ones((128, 128), np.float32),
        # [tk, tq] orientation: invalid where tk > tq
        "trilT": np.tril(np.full((128, 128), NEG, np.float32), -1),
    }
    # token block order: [b0 low-chunk, b1 low, b0 high, b1 high] so block
    # qb = 2*qi + b. qflag row qi selects that qi's 256-token range.
    qf = np.zeros((2, NH, TPC), np.float32)
    for qi in range(2):
        qf[qi, :, qi * 256:(qi + 1) * 256] = 1.0
    shared["qflag"] = qf.astype(bf16)

    in_maps = []
    for c in range(NC):
        pos = []
        for j in core_chunks(c):
            for b in range(B):
                pos.extend((b, j * CH + i) for i in range(CH))
        bidx = np.array([p[0] for p in pos])
        pidx = np.array([p[1] for p in pos])
        x0 = emb[ids[bidx, pidx]]                    # [512, D]
        # x0T[p, k, t] = x0[t, k*128+p]
        x0T = np.ascontiguousarray(
            x0.reshape(TPC, KS, 128).transpose(2, 1, 0))
        cq = np.ascontiguousarray(cos[pidx].T) * scale
        sq = np.ascontiguousarray(sin[pidx].T) * sgn[:, None] * scale
        ck = np.ascontiguousarray(cos[pidx].T)
        sk = np.ascontiguousarray(sin[pidx].T) * sgn[:, None]
        # kbias [half, qi, slot, g, m]: half 0 slot s = chunk s, half 1
        # slot s = chunk 15-s; window chunk valid iff chunk < own chunk j
        kb = np.zeros((2, 2, 8, KVH, 128), np.float32)
        for qi, j in enumerate(core_chunks(c)):
            for X in range(2):
                for s in range(8):
                    ch = s if X == 0 else NCH - 1 - s
                    kb[X, qi, s] = 0.0 if ch < j else NEG
        m = {"x0T": x0T, "cosq": cq.astype(np.float32),
             "sinq": sq.astype(np.float32), "cosk": ck.astype(np.float32),
             "sink": sk.astype(np.float32),
             "kbias": kb.astype(bf16)}
        m.update(shared)
        in_maps.append(m)
    return in_maps


def unshard(results):
    out = np.zeros((B, S, V), np.float32)
    for c in range(NC):
        logits = np.asarray(results[c]["out"], np.float32)
        for qi, j in enumerate(core_chunks(c)):
            for b in range(B):
                qb = 2 * qi + b
                out[b, j * CH:(j + 1) * CH] = logits[qb * 128:(qb + 1) * 128]
    return out


def kernel(**inputs) -> np.ndarray:
    from concourse.bass_utils import run_bass_kernel_spmd
    nc = build_nc()
    in_maps = host_prep(inputs)
    res = run_bass_kernel_spmd(nc, in_maps, core_ids=list(range(NC)),
                               trace=False)
    return unshard(res.results)

